# revision 2
# baseline (speedup 1.0000x reference)
"""Trainium2 Bass kernel for a transformer attention block (BasicBlock), v3.

Reference computation (B=2, L=2048, D=1024, H=16, C=64):
    qkv = x @ w_qkv.T + b_qkv ; q,k,v = split(qkv)
    attn = softmax((q @ k.T) / sqrt(D)) ; heads = attn @ v
    out  = heads @ w_o.T + b_o + x

Sharding: 8 cores = 2 batches x 4 head-groups (4 heads each), bf16 compute.
Per core (b, g), heads processed as 2 pairs (head 2p on partitions 0-63,
head 2p+1 on 64-127):
    P1: qkvT = w_qkv_g @ x_b.T (+bias for q,k at eviction)     [512, 2048] x2
    P2: V    = x_b @ w_v_g per m-tile -> [V_h | 1] blocks      [2048, 4x65]
    scores: 64x128 row-tiled matmul pairs (T0: head 2p from partitions 0-63,
        T8: head 2p+1 from 64-127) -> S^T chunks for BOTH heads
        concurrently, 2x PE throughput at C=64.
    exp: one ACT instruction per [128, 1024] psum tile (fused *SCALE)
    AV: po = [V_h | 1]^T @ P^T -> rows 0..63 = O^T, row 64 = denominators
    normalize: DVE recip -> DMA row to partition 0 -> Pool broadcast ->
        DVE mul.  Even head writes ot in place; odd head staged via DMA.
    P4: out = O @ w_o_g.T per l-chunk (overlaps later attention)

Emission order starts attention for pair 0 as soon as K0/Q0 exist, so the
ACT engine (the exp bottleneck, ~127us busy) starts ~17us in instead of
~67us; scores matmuls are high-priority so ACT never starves at iteration
boundaries.  Host: sum 4 group partials per batch, add x + b_o + w_o @ b_v.
"""

import sys

if "/opt/trn_rl_repo" not in sys.path:
    sys.path.insert(0, "/opt/trn_rl_repo")

import numpy as np

B, L, D, H = 2, 2048, 1024, 16
C = 64
HPC = 4            # heads per core
G = 256            # dims per head group (HPC * C)
SCALE = float(1.0 / np.sqrt(np.float32(D)))

LC = 512           # l-chunk (moving dim)
NLC = L // LC      # 4
MT = L // 128      # 16 m-tiles
DT = D // 128      # 8 d-tiles
NEC = D // 512     # 2 e-chunks for out projection

_CACHE = {}


def _build(reps=1):
    from contextlib import ExitStack

    import concourse.mybir as mybir
    import concourse.tile as tile
    from concourse import bacc

    f32 = mybir.dt.float32
    bf16 = mybir.dt.bfloat16
    Exp = mybir.ActivationFunctionType.Exp

    nc = bacc.Bacc("TRN2", target_bir_lowering=False, debug=False)

    xT = nc.declare_dram_parameter("xT", [D, L], bf16, isOutput=False)
    # columns: [Q (256) | K (256) | V (256)] of this head group, transposed
    wqkvT = nc.declare_dram_parameter("wqkvT", [D, 3 * G], bf16, isOutput=False)
    bqk = nc.declare_dram_parameter("bqk", [128, 4], f32, isOutput=False)
    woT = nc.declare_dram_parameter("woT", [G, D], bf16, isOutput=False)
    out = nc.declare_dram_parameter("out", [L, D], bf16, isOutput=True)

    with tile.TileContext(nc) as tc, nc.allow_low_precision(
        reason="bf16 compute fits the 2e-2 rel-err budget"
    ):
      for _rep in range(reps):
        with (
            tc.tile_pool(name="const", bufs=1) as constp,
            tc.tile_pool(name="qp", bufs=2) as qpp,
            tc.tile_pool(name="kp", bufs=2) as kpp,
            tc.tile_pool(name="vt", bufs=16) as vtp,
            tc.tile_pool(name="wo", bufs=2) as wop,
            tc.tile_pool(name="ot", bufs=8) as otp,
            tc.tile_pool(name="pt", bufs=10) as ptp,
            tc.tile_pool(name="rcp", bufs=3) as rcpp,
            tc.tile_pool(name="nrm", bufs=2) as nrmp,
            tc.tile_pool(name="stg", bufs=3) as stgp,
            tc.tile_pool(name="ps_sc", bufs=2, space="PSUM") as pssc,
            tc.tile_pool(name="ps_o", bufs=2, space="PSUM") as pso,
        ):
            bqk_sb = constp.tile([128, 4], f32)
            nc.sync.dma_start(out=bqk_sb[:], in_=bqk[:])

            wo_sb = []
            for t in range(2):
                w = wop.tile([128, D], bf16, name="wo_sb", tag="wo_sb")
                nc.sync.dma_start(out=w[:], in_=woT[t * 128:(t + 1) * 128, :])
                wo_sb.append(w)

            # qp[p]/kp[p]: Q^T/K^T pair tiles (partitions 0-63 head 2p,
            # 64-127 head 2p+1)
            qp = [qpp.tile([128, L], bf16, name="qp", tag="qp") for _ in range(2)]
            kp = [kpp.tile([128, L], bf16, name="kp", tag="kp") for _ in range(2)]
            # vt[mt]: [128, 4*65]; per head block: [V_h (64 cols) | ones]
            vt = [vtp.tile([128, HPC * 65], bf16, name="vt", tag="vt")
                  for _ in range(MT)]
            # ot[p][lc]: normalized O^T for pair p, l-chunk lc
            ot = [[otp.tile([128, LC], bf16, name="ot", tag="ot")
                   for _ in range(NLC)] for _ in range(2)]

            def p1_chain(xt, wq, t, lc):
                """One QKV projection chain (K or Q pair t, l-chunk lc)."""
                dst = kp[t - 2] if t >= 2 else qp[t]
                ps = psA.tile([128, LC], f32, name="ps", tag="ps")
                for d in range(DT):
                    nc.tensor.matmul(
                        ps[:],
                        lhsT=wq[d][:, t * 128:(t + 1) * 128],
                        rhs=xt[d][:, lc * LC:(lc + 1) * LC],
                        start=(d == 0),
                        stop=(d == DT - 1),
                    )
                ls = slice(lc * LC, (lc + 1) * LC)
                nc.vector.tensor_scalar_add(
                    dst[:, ls], ps[:], bqk_sb[:, t:t + 1]
                )

            def normalize(po, p, lc, parity):
                """rows 0..63 of po = O^T, row 64 = denominators."""
                rc = rcpp.tile([128, LC], f32, name="rc", tag="rc")
                nc.vector.reciprocal(rc[64:65, :], po[64:65, :])
                # partition_broadcast reads physical partition 0; stage the
                # reciprocal row there via a small SBUF DMA
                rc0 = rcpp.tile([1, LC], f32, name="rc0", tag="rc0")
                nc.sync.dma_start(out=rc0[0:1, :], in_=rc[64:65, :])
                rb = rcpp.tile([64, LC], f32, name="rb", tag="rb")
                nc.gpsimd.partition_broadcast(rb[:], rc0[0:1, :])
                if parity == 0:
                    nc.vector.tensor_mul(ot[p][lc][0:64, :], po[0:64, :], rb[:])
                else:
                    nt = nrmp.tile([64, LC], bf16, name="nt")
                    nc.vector.tensor_mul(nt[:], po[0:64, :], rb[:])
                    nc.sync.dma_start(out=ot[p][lc][64:128, :], in_=nt[:])

            def scores_exp(p, lc, j):
                """Row-tiled scores matmul pair + exp for m-tile j; returns
                the bf16 P^T tile (cols 0:LC head 2p, LC:2LC head 2p+1)."""
                ls = slice(lc * LC, (lc + 1) * LC)
                ms = slice(j * 128, (j + 1) * 128)
                ps = pssc.tile([128, 2 * LC], f32)
                with tc.high_priority():
                    nc.tensor.matmul(
                        ps[:, 0:LC],
                        lhsT=kp[p][0:64, ms],
                        rhs=qp[p][0:64, ls],
                        start=True,
                        stop=True,
                    )
                    nc.tensor.matmul(
                        ps[:, LC:2 * LC],
                        lhsT=kp[p][64:128, ms],
                        rhs=qp[p][64:128, ls],
                        start=True,
                        stop=True,
                    )
                pt = ptp.tile([128, 2 * LC], bf16)
                nc.scalar.activation(pt[:], ps[:], Exp, scale=SCALE)
                return pt

            def attention(p, prepassed=()):
                """Attention for head pair p; both heads concurrently via
                64x128 row tiling.  `prepassed` = {lc: P^T tiles} emitted
                ahead of time (so the ACT exp stream starts early)."""
                he, ho = 2 * p, 2 * p + 1
                for lc in range(NLC):
                    po_e = pso.tile([128, LC], f32, name="po", tag="po")
                    po_o = pso.tile([128, LC], f32, name="po", tag="po")
                    for j in range(MT):
                        if lc in prepassed:
                            pt = prepassed[lc][j]
                        else:
                            pt = scores_exp(p, lc, j)
                        nc.tensor.matmul(
                            po_e[0:65, :],
                            lhsT=vt[j][:, he * 65:(he + 1) * 65],
                            rhs=pt[:, 0:LC],
                            start=(j == 0),
                            stop=(j == MT - 1),
                        )
                        nc.tensor.matmul(
                            po_o[0:65, :],
                            lhsT=vt[j][:, ho * 65:(ho + 1) * 65],
                            rhs=pt[:, LC:2 * LC],
                            start=(j == 0),
                            stop=(j == MT - 1),
                        )
                    normalize(po_e, p, lc, 0)
                    normalize(po_o, p, lc, 1)

            _stk = ExitStack()
            xtp = _stk.enter_context(tc.tile_pool(name="xt", bufs=DT))
            wqkvp = _stk.enter_context(tc.tile_pool(name="wqkv", bufs=DT))
            psA = _stk.enter_context(
                tc.tile_pool(name="psA", bufs=2, space="PSUM")
            )

            xt, wq = [], []
            for i in range(DT):
                w = wqkvp.tile([128, 3 * G], bf16, name="wqkv_sb", tag="wqkv_sb")
                nc.sync.dma_start(out=w[:], in_=wqkvT[i * 128:(i + 1) * 128, :])
                wq.append(w)
                xt.append(xtp.tile([128, L], bf16, name="x_sb", tag="x_sb"))
            for c in range(NLC):
                cs = slice(c * LC, (c + 1) * LC)
                for i in range(DT):
                    nc.sync.dma_start(
                        out=xt[i][:, cs], in_=xT[i * 128:(i + 1) * 128, cs]
                    )

            # K0, Q0 first so pair-0 attention (and with it the ACT exp
            # stream) starts as early as possible
            for t in (2, 0):
                for lc in range(NLC):
                    p1_chain(xt, wq, t, lc)

            # ---- P2: V = xT.T @ wv (direct, N=256) ----
            for mt in range(MT):
                ps = psA.tile([128, LC], f32, name="ps", tag="ps")
                for d in range(DT):
                    nc.tensor.matmul(
                        ps[:, 0:G],
                        lhsT=xt[d][:, mt * 128:(mt + 1) * 128],
                        rhs=wq[d][:, 2 * G:3 * G],
                        start=(d == 0),
                        stop=(d == DT - 1),
                    )
                v3d = vt[mt][:].rearrange("p (h c) -> p h c", h=HPC)
                nc.vector.tensor_copy(
                    v3d[:, :, 0:64],
                    ps[:, 0:G].rearrange("p (h c) -> p h c", h=HPC),
                )
                nc.vector.tensor_scalar(
                    v3d[:, :, 64:65], v3d[:, :, 0:1], 0.0, 1.0,
                    mybir.AluOpType.mult, mybir.AluOpType.add,
                )

            attention(0)
            for t in (3, 1):
                for lc in range(NLC):
                    p1_chain(xt, wq, t, lc)
            _stk.close()

            with tc.tile_pool(name="ps4", bufs=2, space="PSUM") as ps4p:

                def p4(lc):
                    """out = O @ woT for one l-chunk."""
                    for lt4 in range(4):
                        lt = lc * 4 + lt4
                        cs = slice(lt4 * 128, (lt4 + 1) * 128)
                        for ec in range(NEC):
                            ps4 = ps4p.tile([128, 512], f32, name="ps4")
                            for t in range(2):
                                nc.tensor.matmul(
                                    ps4[:],
                                    lhsT=ot[t][lc][:, cs],
                                    rhs=wo_sb[t][:, ec * 512:(ec + 1) * 512],
                                    start=(t == 0),
                                    stop=(t == 1),
                                )
                            st = stgp.tile([128, 512], bf16)
                            nc.vector.tensor_copy(st[:], ps4[:])
                            nc.sync.dma_start(
                                out=out[lt * 128:(lt + 1) * 128,
                                        ec * 512:(ec + 1) * 512],
                                in_=st[:],
                            )

                attention(1)
                for lc in range(NLC):
                    p4(lc)

    nc.compile()
    return nc


def _prep_in_maps(x, w_qkv, b_qkv, w_o):
    import ml_dtypes

    bf = ml_dtypes.bfloat16
    xT = [np.ascontiguousarray(x[b].T).astype(bf) for b in range(B)]
    in_maps = []
    for core in range(8):
        b, g = divmod(core, 4)
        qs, ks, vs = g * G, D + g * G, 2 * D + g * G
        wqkvT = np.ascontiguousarray(
            np.concatenate(
                [w_qkv[qs:qs + G], w_qkv[ks:ks + G], w_qkv[vs:vs + G]], axis=0
            ).T
        ).astype(bf)
        bqk_m = np.ascontiguousarray(
            np.concatenate([b_qkv[qs:qs + G], b_qkv[ks:ks + G]]).reshape(4, 128).T
        ).astype(np.float32)
        woT = np.ascontiguousarray(w_o[:, g * G:(g + 1) * G].T).astype(bf)
        in_maps.append({"xT": xT[b], "wqkvT": wqkvT, "bqk": bqk_m, "woT": woT})
    return in_maps


def kernel(x, w_qkv, b_qkv, w_o, b_o):
    from concourse.bass_utils import run_bass_kernel_spmd

    x = np.asarray(x, dtype=np.float32)
    w_qkv = np.asarray(w_qkv, dtype=np.float32)
    b_qkv = np.asarray(b_qkv, dtype=np.float32)
    w_o = np.asarray(w_o, dtype=np.float32)
    b_o = np.asarray(b_o, dtype=np.float32)

    if "nc" not in _CACHE:
        _CACHE["nc"] = _build()
    nc = _CACHE["nc"]

    in_maps = _prep_in_maps(x, w_qkv, b_qkv, w_o)
    res = run_bass_kernel_spmd(nc, in_maps, list(range(8)))
    partial = np.stack(
        [res.results[i]["out"].astype(np.float32) for i in range(8)]
    )  # [8, L, D]

    const = w_o @ b_qkv[2 * D:] + b_o  # [D]
    out = partial.reshape(B, 4, L, D).sum(axis=1) + x + const[None, None, :]
    return out.astype(np.float32)


# revision 6
# speedup vs baseline: 1.0252x; 1.0252x over previous
"""Trainium2 Bass kernel for a transformer attention block (BasicBlock), v3.

Reference computation (B=2, L=2048, D=1024, H=16, C=64):
    qkv = x @ w_qkv.T + b_qkv ; q,k,v = split(qkv)
    attn = softmax((q @ k.T) / sqrt(D)) ; heads = attn @ v
    out  = heads @ w_o.T + b_o + x

Sharding: 8 cores = 2 batches x 4 head-groups (4 heads each), bf16 compute.
Per core (b, g), heads processed as 2 pairs (head 2p on partitions 0-63,
head 2p+1 on 64-127):
    P1: qkvT = w_qkv_g @ x_b.T (+bias for q,k at eviction)     [512, 2048] x2
    P2: V    = x_b @ w_v_g per m-tile -> [V_h | 1] blocks      [2048, 4x65]
    scores: 64x128 row-tiled matmul pairs (T0: head 2p from partitions 0-63,
        T8: head 2p+1 from 64-127) -> S^T chunks for BOTH heads
        concurrently, 2x PE throughput at C=64.
    exp: one ACT instruction per [128, 1024] psum tile (fused *SCALE)
    AV: po = [V_h | 1]^T @ P^T -> rows 0..63 = O^T, row 64 = denominators
    normalize: DVE recip -> DMA row to partition 0 -> Pool broadcast ->
        DVE mul.  Even head writes ot in place; odd head staged via DMA.
    P4: out = O @ w_o_g.T per l-chunk (overlaps later attention)

Emission order starts attention for pair 0 as soon as K0/Q0 exist, so the
ACT engine (the exp bottleneck, ~127us busy) starts ~17us in instead of
~67us; scores matmuls are high-priority so ACT never starves at iteration
boundaries.  Host: sum 4 group partials per batch, add x + b_o + w_o @ b_v.
"""

import sys

if "/opt/trn_rl_repo" not in sys.path:
    sys.path.insert(0, "/opt/trn_rl_repo")

import numpy as np

B, L, D, H = 2, 2048, 1024, 16
C = 64
HPC = 4            # heads per core
G = 256            # dims per head group (HPC * C)
SCALE = float(1.0 / np.sqrt(np.float32(D)))

LC = 512           # l-chunk (moving dim)
NLC = L // LC      # 4
MT = L // 128      # 16 m-tiles
DT = D // 128      # 8 d-tiles
NEC = D // 512     # 2 e-chunks for out projection

_CACHE = {}


def _build(reps=1):
    from contextlib import ExitStack

    import concourse.mybir as mybir
    import concourse.tile as tile
    from concourse import bacc

    f32 = mybir.dt.float32
    bf16 = mybir.dt.bfloat16
    Exp = mybir.ActivationFunctionType.Exp

    nc = bacc.Bacc("TRN2", target_bir_lowering=False, debug=False)

    xT = nc.declare_dram_parameter("xT", [D, L], bf16, isOutput=False)
    # columns: [Q (256) | K (256) | V (256)] of this head group, transposed
    wqkvT = nc.declare_dram_parameter("wqkvT", [D, 3 * G], bf16, isOutput=False)
    bqk = nc.declare_dram_parameter("bqk", [128, 4], f32, isOutput=False)
    woT = nc.declare_dram_parameter("woT", [G, D], bf16, isOutput=False)
    out = nc.declare_dram_parameter("out", [L, D], bf16, isOutput=True)

    with tile.TileContext(nc) as tc, nc.allow_low_precision(
        reason="bf16 compute fits the 2e-2 rel-err budget"
    ):
      for _rep in range(reps):
        with (
            tc.tile_pool(name="const", bufs=1) as constp,
            tc.tile_pool(name="qp", bufs=2) as qpp,
            tc.tile_pool(name="kp", bufs=2) as kpp,
            tc.tile_pool(name="vt", bufs=16) as vtp,
            tc.tile_pool(name="wo", bufs=2) as wop,
            tc.tile_pool(name="ot", bufs=8) as otp,
            tc.tile_pool(name="pt", bufs=10) as ptp,
            tc.tile_pool(name="rcp", bufs=3) as rcpp,
            tc.tile_pool(name="nrm", bufs=2) as nrmp,
            tc.tile_pool(name="stg", bufs=3) as stgp,
            tc.tile_pool(name="ps_sc", bufs=2, space="PSUM") as pssc,
            tc.tile_pool(name="ps_o", bufs=2, space="PSUM") as pso,
        ):
            bqk_sb = constp.tile([128, 4], f32)
            nc.sync.dma_start(out=bqk_sb[:], in_=bqk[:])
            ones_f32 = constp.tile([128, 64], f32)
            nc.vector.memset(ones_f32[:], 1.0)
            ones_bf = constp.tile([128, 64], bf16)
            nc.vector.tensor_scalar(
                ones_bf[:], ones_f32[:], 0.0, 1.0,
                mybir.AluOpType.mult, mybir.AluOpType.add,
            )

            wo_sb = []
            for t in range(2):
                w = wop.tile([128, D], bf16, name="wo_sb", tag="wo_sb")
                nc.sync.dma_start(out=w[:], in_=woT[t * 128:(t + 1) * 128, :])
                wo_sb.append(w)

            # qp[p]/kp[p]: Q^T/K^T pair tiles (partitions 0-63 head 2p,
            # 64-127 head 2p+1)
            qp = [qpp.tile([128, L], bf16, name="qp", tag="qp") for _ in range(2)]
            kp = [kpp.tile([128, L], bf16, name="kp", tag="kp") for _ in range(2)]
            # vt[mt]: [128, 4*65]; per head block: [V_h (64 cols) | ones]
            vt = [vtp.tile([128, HPC * 65], bf16, name="vt", tag="vt")
                  for _ in range(MT)]
            # ot[p][lc]: normalized O^T for pair p, l-chunk lc
            ot = [[otp.tile([128, LC], bf16, name="ot", tag="ot")
                   for _ in range(NLC)] for _ in range(2)]

            def p1_chain(xt, wq, t, lc):
                """One QKV projection chain (K or Q pair t, l-chunk lc)."""
                dst = kp[t - 2] if t >= 2 else qp[t]
                ps = psA.tile([128, LC], f32, name="ps", tag="ps")
                for d in range(DT):
                    nc.tensor.matmul(
                        ps[:],
                        lhsT=wq[d][:, t * 128:(t + 1) * 128],
                        rhs=xt[d][:, lc * LC:(lc + 1) * LC],
                        start=(d == 0),
                        stop=(d == DT - 1),
                    )
                ls = slice(lc * LC, (lc + 1) * LC)
                nc.vector.tensor_scalar_add(
                    dst[:, ls], ps[:], bqk_sb[:, t:t + 1]
                )

            def normalize(po, p, lc, parity):
                """rows 0..63 of po = O^T, row 64 = denominators."""
                rc = rcpp.tile([128, LC], f32, name="rc", tag="rc")
                nc.vector.reciprocal(rc[64:65, :], po[64:65, :])
                # partition_broadcast reads physical partition 0; stage the
                # reciprocal row there via a small SBUF DMA
                rc0 = rcpp.tile([1, LC], f32, name="rc0", tag="rc0")
                nc.sync.dma_start(out=rc0[0:1, :], in_=rc[64:65, :])
                rb = rcpp.tile([64, LC], f32, name="rb", tag="rb")
                nc.gpsimd.partition_broadcast(rb[:], rc0[0:1, :])
                if parity == 0:
                    nc.vector.tensor_mul(ot[p][lc][0:64, :], po[0:64, :], rb[:])
                else:
                    nt = nrmp.tile([64, LC], bf16, name="nt")
                    nc.vector.tensor_mul(nt[:], po[0:64, :], rb[:])
                    nc.sync.dma_start(out=ot[p][lc][64:128, :], in_=nt[:])

            def scores_exp(p, lc, j):
                """Row-tiled scores matmul pair + exp for m-tile j; returns
                the bf16 P^T tile (cols 0:LC head 2p, LC:2LC head 2p+1)."""
                ls = slice(lc * LC, (lc + 1) * LC)
                ms = slice(j * 128, (j + 1) * 128)
                ps = pssc.tile([128, 2 * LC], f32, name="sc", tag="sc")
                with tc.high_priority():
                    nc.tensor.matmul(
                        ps[:, 0:LC],
                        lhsT=kp[p][0:64, ms],
                        rhs=qp[p][0:64, ls],
                        start=True,
                        stop=True,
                    )
                    nc.tensor.matmul(
                        ps[:, LC:2 * LC],
                        lhsT=kp[p][64:128, ms],
                        rhs=qp[p][64:128, ls],
                        start=True,
                        stop=True,
                    )
                pt = ptp.tile([128, 2 * LC], bf16)
                nc.scalar.activation(pt[:], ps[:], Exp, scale=SCALE)
                return pt

            def attention(p, prepassed=()):
                """Attention for head pair p; both heads concurrently via
                64x128 row tiling.  `prepassed` = {lc: P^T tiles} emitted
                ahead of time (so the ACT exp stream starts early)."""
                he, ho = 2 * p, 2 * p + 1
                for lc in range(NLC):
                    po_e = pso.tile([128, LC], f32, name="po", tag="po")
                    po_o = pso.tile([128, LC], f32, name="po", tag="po")
                    for j in range(MT):
                        if lc in prepassed:
                            pt = prepassed[lc][j]
                        else:
                            pt = scores_exp(p, lc, j)
                        nc.tensor.matmul(
                            po_e[0:65, :],
                            lhsT=vt[j][:, he * 65:(he + 1) * 65],
                            rhs=pt[:, 0:LC],
                            start=(j == 0),
                            stop=(j == MT - 1),
                        )
                        nc.tensor.matmul(
                            po_o[0:65, :],
                            lhsT=vt[j][:, ho * 65:(ho + 1) * 65],
                            rhs=pt[:, LC:2 * LC],
                            start=(j == 0),
                            stop=(j == MT - 1),
                        )
                    normalize(po_e, p, lc, 0)
                    normalize(po_o, p, lc, 1)

            _stk = ExitStack()
            xtp = _stk.enter_context(tc.tile_pool(name="xt", bufs=DT))
            wqkvp = _stk.enter_context(tc.tile_pool(name="wqkv", bufs=DT))
            psA = _stk.enter_context(
                tc.tile_pool(name="psA", bufs=2, space="PSUM")
            )

            # warm the PE clock gate (HAM) during the load window so the
            # K0/Q0 chains — which gate the first exp — run at full clock
            wps = pssc.tile([128, 2 * LC], f32, name="sc", tag="sc")
            for _ in range(48):
                nc.tensor.matmul(
                    wps[0:64, 0:64], lhsT=ones_bf[:, 0:64],
                    rhs=ones_bf[:, 0:64], start=True, stop=True,
                )
            nc.vector.tensor_copy(ones_f32[0:8, 0:8], wps[0:8, 0:8])

            # interleave wq[d] with x[d] lc0 so the first P1 chains pipeline
            # directly behind the loads
            xt, wq = [], []
            for i in range(DT):
                w = wqkvp.tile([128, 3 * G], bf16, name="wqkv_sb", tag="wqkv_sb")
                nc.sync.dma_start(out=w[:], in_=wqkvT[i * 128:(i + 1) * 128, :])
                wq.append(w)
                x_sb = xtp.tile([128, L], bf16, name="x_sb", tag="x_sb")
                nc.sync.dma_start(
                    out=x_sb[:, 0:LC], in_=xT[i * 128:(i + 1) * 128, 0:LC]
                )
                xt.append(x_sb)
            for c in range(1, NLC):
                cs = slice(c * LC, (c + 1) * LC)
                for i in range(DT):
                    nc.sync.dma_start(
                        out=xt[i][:, cs], in_=xT[i * 128:(i + 1) * 128, cs]
                    )

            # K0/Q0 interleaved by l-chunk so Q0-lc0 (which the first
            # scores need) lands as the second chain, not the fifth
            for lc in range(NLC):
                p1_chain(xt, wq, 2, lc)
                p1_chain(xt, wq, 0, lc)

            # ---- P2: V = xT.T @ wv (direct, N=256) ----
            for mt in range(MT):
                ps = psA.tile([128, LC], f32, name="ps", tag="ps")
                for d in range(DT):
                    nc.tensor.matmul(
                        ps[:, 0:G],
                        lhsT=xt[d][:, mt * 128:(mt + 1) * 128],
                        rhs=wq[d][:, 2 * G:3 * G],
                        start=(d == 0),
                        stop=(d == DT - 1),
                    )
                v3d = vt[mt][:].rearrange("p (h c) -> p h c", h=HPC)
                nc.vector.tensor_copy(
                    v3d[:, :, 0:64],
                    ps[:, 0:G].rearrange("p (h c) -> p h c", h=HPC),
                )
                nc.vector.tensor_scalar(
                    v3d[:, :, 64:65], v3d[:, :, 0:1], 0.0, 1.0,
                    mybir.AluOpType.mult, mybir.AluOpType.add,
                )

            attention(0)
            for t in (3, 1):
                for lc in range(NLC):
                    p1_chain(xt, wq, t, lc)
            _stk.close()

            with tc.tile_pool(name="ps4", bufs=2, space="PSUM") as ps4p:

                def p4(lc):
                    """out = O @ woT for one l-chunk."""
                    for lt4 in range(4):
                        lt = lc * 4 + lt4
                        cs = slice(lt4 * 128, (lt4 + 1) * 128)
                        for ec in range(NEC):
                            ps4 = ps4p.tile([128, 512], f32, name="ps4")
                            for t in range(2):
                                nc.tensor.matmul(
                                    ps4[:],
                                    lhsT=ot[t][lc][:, cs],
                                    rhs=wo_sb[t][:, ec * 512:(ec + 1) * 512],
                                    start=(t == 0),
                                    stop=(t == 1),
                                )
                            st = stgp.tile([128, 512], bf16)
                            nc.vector.tensor_copy(st[:], ps4[:])
                            nc.sync.dma_start(
                                out=out[lt * 128:(lt + 1) * 128,
                                        ec * 512:(ec + 1) * 512],
                                in_=st[:],
                            )

                attention(1)
                for lc in range(NLC):
                    p4(lc)

    nc.compile()
    return nc


def _prep_in_maps(x, w_qkv, b_qkv, w_o):
    import ml_dtypes

    bf = ml_dtypes.bfloat16
    xT = [np.ascontiguousarray(x[b].T).astype(bf) for b in range(B)]
    in_maps = []
    for core in range(8):
        b, g = divmod(core, 4)
        qs, ks, vs = g * G, D + g * G, 2 * D + g * G
        wqkvT = np.ascontiguousarray(
            np.concatenate(
                [w_qkv[qs:qs + G], w_qkv[ks:ks + G], w_qkv[vs:vs + G]], axis=0
            ).T
        ).astype(bf)
        bqk_m = np.ascontiguousarray(
            np.concatenate([b_qkv[qs:qs + G], b_qkv[ks:ks + G]]).reshape(4, 128).T
        ).astype(np.float32)
        woT = np.ascontiguousarray(w_o[:, g * G:(g + 1) * G].T).astype(bf)
        in_maps.append({"xT": xT[b], "wqkvT": wqkvT, "bqk": bqk_m, "woT": woT})
    return in_maps


def kernel(x, w_qkv, b_qkv, w_o, b_o):
    from concourse.bass_utils import run_bass_kernel_spmd

    x = np.asarray(x, dtype=np.float32)
    w_qkv = np.asarray(w_qkv, dtype=np.float32)
    b_qkv = np.asarray(b_qkv, dtype=np.float32)
    w_o = np.asarray(w_o, dtype=np.float32)
    b_o = np.asarray(b_o, dtype=np.float32)

    if "nc" not in _CACHE:
        _CACHE["nc"] = _build()
    nc = _CACHE["nc"]

    in_maps = _prep_in_maps(x, w_qkv, b_qkv, w_o)
    res = run_bass_kernel_spmd(nc, in_maps, list(range(8)))
    partial = np.stack(
        [res.results[i]["out"].astype(np.float32) for i in range(8)]
    )  # [8, L, D]

    const = w_o @ b_qkv[2 * D:] + b_o  # [D]
    out = partial.reshape(B, 4, L, D).sum(axis=1) + x + const[None, None, :]
    return out.astype(np.float32)


# revision 7
# speedup vs baseline: 1.0769x; 1.0505x over previous
"""Trainium2 Bass kernel for a transformer attention block (BasicBlock), v3.

Reference computation (B=2, L=2048, D=1024, H=16, C=64):
    qkv = x @ w_qkv.T + b_qkv ; q,k,v = split(qkv)
    attn = softmax((q @ k.T) / sqrt(D)) ; heads = attn @ v
    out  = heads @ w_o.T + b_o + x

Sharding: 8 cores = 2 batches x 4 head-groups (4 heads each), bf16 compute.
Per core (b, g), heads processed as 2 pairs (head 2p on partitions 0-63,
head 2p+1 on 64-127):
    P1: qkvT = w_qkv_g @ x_b.T (+bias for q,k at eviction)     [512, 2048] x2
    P2: V    = x_b @ w_v_g per m-tile -> [V_h | 1] blocks      [2048, 4x65]
    scores: 64x128 row-tiled matmul pairs (T0: head 2p from partitions 0-63,
        T8: head 2p+1 from 64-127) -> S^T chunks for BOTH heads
        concurrently, 2x PE throughput at C=64.
    exp: one ACT instruction per [128, 1024] psum tile (fused *SCALE)
    AV: po = [V_h | 1]^T @ P^T -> rows 0..63 = O^T, row 64 = denominators
    normalize: DVE recip -> DMA row to partition 0 -> Pool broadcast ->
        DVE mul.  Even head writes ot in place; odd head staged via DMA.
    P4: out = O @ w_o_g.T per l-chunk (overlaps later attention)

Emission order starts attention for pair 0 as soon as K0/Q0 exist, so the
ACT engine (the exp bottleneck, ~127us busy) starts ~17us in instead of
~67us; scores matmuls are high-priority so ACT never starves at iteration
boundaries.  Host: sum 4 group partials per batch, add x + b_o + w_o @ b_v.
"""

import sys

if "/opt/trn_rl_repo" not in sys.path:
    sys.path.insert(0, "/opt/trn_rl_repo")

import numpy as np

B, L, D, H = 2, 2048, 1024, 16
C = 64
HPC = 4            # heads per core
G = 256            # dims per head group (HPC * C)
SCALE = float(1.0 / np.sqrt(np.float32(D)))

LC = 512           # l-chunk (moving dim)
NLC = L // LC      # 4
MT = L // 128      # 16 m-tiles
DT = D // 128      # 8 d-tiles
NEC = D // 512     # 2 e-chunks for out projection

_CACHE = {}


def _build(reps=1):
    from contextlib import ExitStack

    import concourse.mybir as mybir
    import concourse.tile as tile
    from concourse import bacc

    f32 = mybir.dt.float32
    bf16 = mybir.dt.bfloat16
    Exp = mybir.ActivationFunctionType.Exp

    nc = bacc.Bacc("TRN2", target_bir_lowering=False, debug=False)

    xT = nc.declare_dram_parameter("xT", [D, L], bf16, isOutput=False)
    # columns: [Q (256) | K (256) | V (256)] of this head group, transposed
    wqkvT = nc.declare_dram_parameter("wqkvT", [D, 3 * G], bf16, isOutput=False)
    bqk = nc.declare_dram_parameter("bqk", [128, 4], f32, isOutput=False)
    woT = nc.declare_dram_parameter("woT", [G, D], bf16, isOutput=False)
    out = nc.declare_dram_parameter("out", [L, D], bf16, isOutput=True)

    with tile.TileContext(nc) as tc, nc.allow_low_precision(
        reason="bf16 compute fits the 2e-2 rel-err budget"
    ):
      for _rep in range(reps):
        with (
            tc.tile_pool(name="const", bufs=1) as constp,
            tc.tile_pool(name="qp", bufs=2) as qpp,
            tc.tile_pool(name="kp", bufs=2) as kpp,
            tc.tile_pool(name="vt", bufs=16) as vtp,
            tc.tile_pool(name="wo", bufs=2) as wop,
            tc.tile_pool(name="ot", bufs=8) as otp,
            tc.tile_pool(name="pt", bufs=14) as ptp,
            tc.tile_pool(name="rcp", bufs=4) as rcpp,
            tc.tile_pool(name="nrm", bufs=3) as nrmp,
            tc.tile_pool(name="stg", bufs=4) as stgp,
            tc.tile_pool(name="ps_sc", bufs=2, space="PSUM") as pssc,
            tc.tile_pool(name="ps_o", bufs=2, space="PSUM") as pso,
        ):
            bqk_sb = constp.tile([128, 4], f32)
            nc.sync.dma_start(out=bqk_sb[:], in_=bqk[:])
            ones_f32 = constp.tile([128, 64], f32)
            nc.vector.memset(ones_f32[:], 1.0)
            ones_bf = constp.tile([128, 64], bf16)
            nc.vector.tensor_scalar(
                ones_bf[:], ones_f32[:], 0.0, 1.0,
                mybir.AluOpType.mult, mybir.AluOpType.add,
            )

            wo_sb = []
            for t in range(2):
                w = wop.tile([128, D], bf16, name="wo_sb", tag="wo_sb")
                nc.sync.dma_start(out=w[:], in_=woT[t * 128:(t + 1) * 128, :])
                wo_sb.append(w)

            # qp[p]/kp[p]: Q^T/K^T pair tiles (partitions 0-63 head 2p,
            # 64-127 head 2p+1)
            qp = [qpp.tile([128, L], bf16, name="qp", tag="qp") for _ in range(2)]
            kp = [kpp.tile([128, L], bf16, name="kp", tag="kp") for _ in range(2)]
            # vt[mt]: [128, 4*65]; per head block: [V_h (64 cols) | ones]
            vt = [vtp.tile([128, HPC * 65], bf16, name="vt", tag="vt")
                  for _ in range(MT)]
            # ot[p][lc]: normalized O^T for pair p, l-chunk lc
            ot = [[otp.tile([128, LC], bf16, name="ot", tag="ot")
                   for _ in range(NLC)] for _ in range(2)]

            def p1_chain(xt, wq, t, lc):
                """One QKV projection chain (K or Q pair t, l-chunk lc)."""
                dst = kp[t - 2] if t >= 2 else qp[t]
                ps = psA.tile([128, LC], f32, name="ps", tag="ps")
                for d in range(DT):
                    nc.tensor.matmul(
                        ps[:],
                        lhsT=wq[d][:, t * 128:(t + 1) * 128],
                        rhs=xt[d][:, lc * LC:(lc + 1) * LC],
                        start=(d == 0),
                        stop=(d == DT - 1),
                    )
                ls = slice(lc * LC, (lc + 1) * LC)
                nc.vector.tensor_scalar_add(
                    dst[:, ls], ps[:], bqk_sb[:, t:t + 1]
                )

            def normalize(po, p, lc, parity):
                """rows 0..63 of po = O^T, row 64 = denominators."""
                rc = rcpp.tile([128, LC], f32, name="rc", tag="rc")
                nc.vector.reciprocal(rc[64:65, :], po[64:65, :])
                # partition_broadcast reads physical partition 0; stage the
                # reciprocal row there via a small SBUF DMA
                rc0 = rcpp.tile([1, LC], f32, name="rc0", tag="rc0")
                nc.sync.dma_start(out=rc0[0:1, :], in_=rc[64:65, :])
                rb = rcpp.tile([64, LC], f32, name="rb", tag="rb")
                nc.gpsimd.partition_broadcast(rb[:], rc0[0:1, :])
                if parity == 0:
                    nc.vector.tensor_mul(ot[p][lc][0:64, :], po[0:64, :], rb[:])
                else:
                    nt = nrmp.tile([64, LC], bf16, name="nt")
                    nc.vector.tensor_mul(nt[:], po[0:64, :], rb[:])
                    nc.sync.dma_start(out=ot[p][lc][64:128, :], in_=nt[:])

            def scores_exp(p, lc, j):
                """Row-tiled scores matmul pair + exp for m-tile j; returns
                the bf16 P^T tile (cols 0:LC head 2p, LC:2LC head 2p+1)."""
                ls = slice(lc * LC, (lc + 1) * LC)
                ms = slice(j * 128, (j + 1) * 128)
                ps = pssc.tile([128, 2 * LC], f32, name="sc", tag="sc")
                with tc.high_priority():
                    nc.tensor.matmul(
                        ps[:, 0:LC],
                        lhsT=kp[p][0:64, ms],
                        rhs=qp[p][0:64, ls],
                        start=True,
                        stop=True,
                    )
                    nc.tensor.matmul(
                        ps[:, LC:2 * LC],
                        lhsT=kp[p][64:128, ms],
                        rhs=qp[p][64:128, ls],
                        start=True,
                        stop=True,
                    )
                pt = ptp.tile([128, 2 * LC], bf16)
                nc.scalar.activation(pt[:], ps[:], Exp, scale=SCALE)
                return pt

            def attention(p, prepassed=()):
                """Attention for head pair p; both heads concurrently via
                64x128 row tiling.  `prepassed` = {lc: P^T tiles} emitted
                ahead of time (so the ACT exp stream starts early)."""
                he, ho = 2 * p, 2 * p + 1
                for lc in range(NLC):
                    po_e = pso.tile([128, LC], f32, name="po", tag="po")
                    po_o = pso.tile([128, LC], f32, name="po", tag="po")
                    for j in range(MT):
                        if lc in prepassed:
                            pt = prepassed[lc][j]
                        else:
                            pt = scores_exp(p, lc, j)
                        nc.tensor.matmul(
                            po_e[0:65, :],
                            lhsT=vt[j][:, he * 65:(he + 1) * 65],
                            rhs=pt[:, 0:LC],
                            start=(j == 0),
                            stop=(j == MT - 1),
                        )
                        nc.tensor.matmul(
                            po_o[0:65, :],
                            lhsT=vt[j][:, ho * 65:(ho + 1) * 65],
                            rhs=pt[:, LC:2 * LC],
                            start=(j == 0),
                            stop=(j == MT - 1),
                        )
                    normalize(po_e, p, lc, 0)
                    normalize(po_o, p, lc, 1)

            _stk = ExitStack()
            xtp = _stk.enter_context(tc.tile_pool(name="xt", bufs=DT))
            wqkvp = _stk.enter_context(tc.tile_pool(name="wqkv", bufs=DT))
            psA = _stk.enter_context(
                tc.tile_pool(name="psA", bufs=2, space="PSUM")
            )

            # warm the PE clock gate (HAM) during the load window so the
            # K0/Q0 chains — which gate the first exp — run at full clock
            wps = pssc.tile([128, 2 * LC], f32, name="sc", tag="sc")
            for _ in range(48):
                nc.tensor.matmul(
                    wps[0:64, 0:64], lhsT=ones_bf[:, 0:64],
                    rhs=ones_bf[:, 0:64], start=True, stop=True,
                )
            nc.vector.tensor_copy(ones_f32[0:8, 0:8], wps[0:8, 0:8])

            # interleave wq[d] with x[d] lc0 so the first P1 chains pipeline
            # directly behind the loads
            xt, wq = [], []
            for i in range(DT):
                w = wqkvp.tile([128, 3 * G], bf16, name="wqkv_sb", tag="wqkv_sb")
                nc.sync.dma_start(out=w[:], in_=wqkvT[i * 128:(i + 1) * 128, :])
                wq.append(w)
                x_sb = xtp.tile([128, L], bf16, name="x_sb", tag="x_sb")
                nc.sync.dma_start(
                    out=x_sb[:, 0:LC], in_=xT[i * 128:(i + 1) * 128, 0:LC]
                )
                xt.append(x_sb)
            for c in range(1, NLC):
                cs = slice(c * LC, (c + 1) * LC)
                for i in range(DT):
                    nc.sync.dma_start(
                        out=xt[i][:, cs], in_=xT[i * 128:(i + 1) * 128, cs]
                    )

            # K0/Q0 interleaved by l-chunk so Q0-lc0 (which the first
            # scores need) lands as the second chain, not the fifth
            for lc in range(NLC):
                p1_chain(xt, wq, 2, lc)
                p1_chain(xt, wq, 0, lc)

            # ---- P2: V = xT.T @ wv (direct, N=256) ----
            for mt in range(MT):
                ps = psA.tile([128, LC], f32, name="ps", tag="ps")
                for d in range(DT):
                    nc.tensor.matmul(
                        ps[:, 0:G],
                        lhsT=xt[d][:, mt * 128:(mt + 1) * 128],
                        rhs=wq[d][:, 2 * G:3 * G],
                        start=(d == 0),
                        stop=(d == DT - 1),
                    )
                v3d = vt[mt][:].rearrange("p (h c) -> p h c", h=HPC)
                nc.vector.tensor_copy(
                    v3d[:, :, 0:64],
                    ps[:, 0:G].rearrange("p (h c) -> p h c", h=HPC),
                )
                nc.vector.tensor_scalar(
                    v3d[:, :, 64:65], v3d[:, :, 0:1], 0.0, 1.0,
                    mybir.AluOpType.mult, mybir.AluOpType.add,
                )

            attention(0)
            for t in (3, 1):
                for lc in range(NLC):
                    p1_chain(xt, wq, t, lc)
            _stk.close()

            with tc.tile_pool(name="ps4", bufs=2, space="PSUM") as ps4p:

                def p4(lc):
                    """out = O @ woT for one l-chunk."""
                    for lt4 in range(4):
                        lt = lc * 4 + lt4
                        cs = slice(lt4 * 128, (lt4 + 1) * 128)
                        for ec in range(NEC):
                            ps4 = ps4p.tile([128, 512], f32, name="ps4")
                            for t in range(2):
                                nc.tensor.matmul(
                                    ps4[:],
                                    lhsT=ot[t][lc][:, cs],
                                    rhs=wo_sb[t][:, ec * 512:(ec + 1) * 512],
                                    start=(t == 0),
                                    stop=(t == 1),
                                )
                            st = stgp.tile([128, 512], bf16)
                            nc.vector.tensor_copy(st[:], ps4[:])
                            nc.sync.dma_start(
                                out=out[lt * 128:(lt + 1) * 128,
                                        ec * 512:(ec + 1) * 512],
                                in_=st[:],
                            )

                attention(1)
                for lc in range(NLC):
                    p4(lc)

    nc.compile()
    return nc


def _prep_in_maps(x, w_qkv, b_qkv, w_o):
    import ml_dtypes

    bf = ml_dtypes.bfloat16
    xT = [np.ascontiguousarray(x[b].T).astype(bf) for b in range(B)]
    in_maps = []
    for core in range(8):
        b, g = divmod(core, 4)
        qs, ks, vs = g * G, D + g * G, 2 * D + g * G
        wqkvT = np.ascontiguousarray(
            np.concatenate(
                [w_qkv[qs:qs + G], w_qkv[ks:ks + G], w_qkv[vs:vs + G]], axis=0
            ).T
        ).astype(bf)
        bqk_m = np.ascontiguousarray(
            np.concatenate([b_qkv[qs:qs + G], b_qkv[ks:ks + G]]).reshape(4, 128).T
        ).astype(np.float32)
        woT = np.ascontiguousarray(w_o[:, g * G:(g + 1) * G].T).astype(bf)
        in_maps.append({"xT": xT[b], "wqkvT": wqkvT, "bqk": bqk_m, "woT": woT})
    return in_maps


def kernel(x, w_qkv, b_qkv, w_o, b_o):
    from concourse.bass_utils import run_bass_kernel_spmd

    x = np.asarray(x, dtype=np.float32)
    w_qkv = np.asarray(w_qkv, dtype=np.float32)
    b_qkv = np.asarray(b_qkv, dtype=np.float32)
    w_o = np.asarray(w_o, dtype=np.float32)
    b_o = np.asarray(b_o, dtype=np.float32)

    if "nc" not in _CACHE:
        _CACHE["nc"] = _build()
    nc = _CACHE["nc"]

    in_maps = _prep_in_maps(x, w_qkv, b_qkv, w_o)
    res = run_bass_kernel_spmd(nc, in_maps, list(range(8)))
    partial = np.stack(
        [res.results[i]["out"].astype(np.float32) for i in range(8)]
    )  # [8, L, D]

    const = w_o @ b_qkv[2 * D:] + b_o  # [D]
    out = partial.reshape(B, 4, L, D).sum(axis=1) + x + const[None, None, :]
    return out.astype(np.float32)
